# revision 3
# baseline (speedup 1.0000x reference)
"""Trainium2 Bass kernel for NodeUpdateGRU: mean over 200000 parent states
followed by a GRUCell update with h=0.

Math (h = 0 simplifies the torch GRUCell):
    agg  = mean(parent_states, axis=0)                  # (512,)
    gi   = weight_ih @ agg + bias_ih                    # (1536,)
    gh   = bias_hh                                      # weight_hh @ 0 == 0
    r    = sigmoid(gi_r + gh_r)
    z    = sigmoid(gi_z + gh_z)
    n    = tanh(gi_n + r * gh_n)
    h'   = (1 - z) * n

Distribution: shard parent_states by rows across 8 cores (contiguous
25000-row slices). Each core computes a partial column sum with the tensor
engine (ones^T @ tile accumulated in PSUM), scales by 1/200000, then a 2KB
AllReduce produces the global mean on every core. The GRU epilogue runs
redundantly on every core; core 0's output is returned.
"""

import sys

for _p in (
    "/root/.axon_site",
    "/root/.axon_site/_ro/trn_rl_repo",
    "/root/.axon_site/_ro/pypackages",
    "/opt/trn_rl_repo",
    "/opt/pypackages",
):
    if _p not in sys.path:
        sys.path.append(_p)

import numpy as np

N_CORES = 8
N_ROWS_TOTAL = 200000
D = 512              # input dim
H = 512              # hidden dim
P = 128              # SBUF partitions
ROWS = N_ROWS_TOTAL // N_CORES   # 25000 rows per core
R = 15               # rows per partition per streaming group
GROUP_ROWS = P * R   # 1920
NC12 = (3 * H) // P  # 12 column-chunks of the 1536-row weight matrix


def build_nc(rows=ROWS, r_per_part=R):
    from concourse import bacc, mybir, tile

    f32 = mybir.dt.float32
    group_rows = P * r_per_part
    n_groups = rows // group_rows
    ragged = rows - n_groups * group_rows
    assert ragged < P

    nc = bacc.Bacc(
        "TRN2", target_bir_lowering=False, debug=False, num_devices=N_CORES
    )

    x = nc.dram_tensor("x", [rows, D], f32, kind="ExternalInput")
    # weight_ih pre-arranged on host: w[p, c*D + d] = W[c*128 + p, d]
    w = nc.dram_tensor("w", [P, NC12 * D], f32, kind="ExternalInput")
    # biases pre-arranged on host: b[p, c] = bias[c*128 + p]
    bih = nc.dram_tensor("bih", [P, NC12], f32, kind="ExternalInput")
    bhh = nc.dram_tensor("bhh", [P, NC12], f32, kind="ExternalInput")
    # output h' arranged as hout[p, c] = h[c*128 + p]
    hout = nc.dram_tensor("hout", [P, 4], f32, kind="ExternalOutput")

    sig = mybir.ActivationFunctionType.Sigmoid
    tanh = mybir.ActivationFunctionType.Tanh
    copy_f = mybir.ActivationFunctionType.Copy

    with tile.TileContext(nc) as tc:
        with (
            tc.tile_pool(name="stream", bufs=3) as stream_pool,
            tc.tile_pool(name="consts", bufs=1) as consts,
            tc.tile_pool(name="psum", bufs=1, space="PSUM") as psum_pool,
            tc.tile_pool(name="dram", bufs=1, space="DRAM") as dram_pool,
            tc.tile_pool(name="small", bufs=2) as small,
        ):
            ones = consts.tile([P, 1], f32)
            nc.vector.memset(ones[:], 1.0)

            w_sb = consts.tile([P, NC12 * D], f32)
            nc.sync.dma_start(out=w_sb[:], in_=w[:, :])
            bih_sb = consts.tile([P, NC12], f32)
            nc.sync.dma_start(out=bih_sb[:], in_=bih[:, :])
            bhh_sb = consts.tile([P, NC12], f32)
            nc.sync.dma_start(out=bhh_sb[:], in_=bhh[:, :])
            # bsum: init value for the W@agg reduction. r/z chunks get
            # bias_ih + bias_hh; n chunks get bias_ih only (bias_hh_n is
            # gated by r later).
            bsum = consts.tile([P, NC12], f32)
            nc.vector.tensor_copy(out=bsum[:], in_=bih_sb[:])
            nc.vector.tensor_add(
                out=bsum[:, 0:8], in0=bsum[:, 0:8], in1=bhh_sb[:, 0:8]
            )

            # ---- streaming phase: column sums into PSUM ----
            psum_acc = psum_pool.tile([1, D], f32)
            n_mm = n_groups * r_per_part + (1 if ragged else 0)
            mm_i = 0
            for g in range(n_groups):
                t = stream_pool.tile([P, r_per_part * D], f32, tag="xtile")
                src = x[g * group_rows : (g + 1) * group_rows, :].rearrange(
                    "(p t) d -> p (t d)", p=P
                )
                nc.sync.dma_start(out=t[:], in_=src)
                for s in range(r_per_part):
                    nc.tensor.matmul(
                        out=psum_acc[:, :],
                        lhsT=ones[:, :],
                        rhs=t[:, s * D : (s + 1) * D],
                        start=(mm_i == 0),
                        stop=(mm_i == n_mm - 1),
                    )
                    mm_i += 1
            if ragged:
                t = stream_pool.tile([P, r_per_part * D], f32, tag="xtile")
                nc.sync.dma_start(
                    out=t[:ragged, 0:D], in_=x[n_groups * group_rows : rows, :]
                )
                nc.tensor.matmul(
                    out=psum_acc[:, :],
                    lhsT=ones[:ragged, :],
                    rhs=t[:ragged, 0:D],
                    start=(mm_i == 0),
                    stop=True,
                )
                mm_i += 1

            # local partial mean (scale by 1/N while copying out of PSUM)
            partial = small.tile([1, D], f32)
            nc.scalar.activation(
                out=partial[:],
                in_=psum_acc[:],
                func=copy_f,
                scale=1.0 / float(N_ROWS_TOTAL),
            )

            # ---- cross-core all-reduce of the 2KB partial mean ----
            cc_in = dram_pool.tile([1, D], f32)
            cc_out = dram_pool.tile([1, D], f32)
            nc.gpsimd.dma_start(out=cc_in[:], in_=partial[:])
            nc.gpsimd.collective_compute(
                "AllReduce",
                mybir.AluOpType.add,
                replica_groups=[list(range(N_CORES))],
                ins=[cc_in[:].opt()],
                outs=[cc_out[:].opt()],
            )
            # broadcast the global mean to all 128 partitions
            agg_b = consts.tile([P, D], f32)
            nc.sync.dma_start(out=agg_b[:], in_=cc_out[:].to_broadcast((P, D)))

            # ---- GRU epilogue ----
            # gi[:, c] = sum_d W[c*128+p, d] * agg[d] + bsum[p, c]
            # (tensor_tensor_reduce would fuse this but crashes the DVE on
            # this runtime build — use mul + reduce instead)
            gi = small.tile([P, NC12], f32)
            for c in range(NC12):
                prod = small.tile([P, D], f32, tag="prod")
                nc.vector.tensor_mul(
                    out=prod[:], in0=w_sb[:, c * D : (c + 1) * D], in1=agg_b[:]
                )
                nc.vector.reduce_sum(
                    out=gi[:, c : c + 1], in_=prod[:], axis=mybir.AxisListType.X
                )
            nc.vector.tensor_add(out=gi[:], in0=gi[:], in1=bsum[:])

            rz = small.tile([P, 8], f32)
            nc.scalar.activation(out=rz[:], in_=gi[:, 0:8], func=sig)
            pre_n = small.tile([P, 4], f32)
            nc.vector.tensor_mul(out=pre_n[:], in0=rz[:, 0:4], in1=bhh_sb[:, 8:12])
            nc.vector.tensor_add(out=pre_n[:], in0=pre_n[:], in1=gi[:, 8:12])
            n_t = small.tile([P, 4], f32)
            nc.scalar.activation(out=n_t[:], in_=pre_n[:], func=tanh)
            zn = small.tile([P, 4], f32)
            nc.vector.tensor_mul(out=zn[:], in0=rz[:, 4:8], in1=n_t[:])
            hres = small.tile([P, 4], f32)
            nc.vector.tensor_sub(out=hres[:], in0=n_t[:], in1=zn[:])
            nc.sync.dma_start(out=hout[:, :], in_=hres[:])

    nc.compile()
    return nc


_NC_CACHE = {}


def _get_nc(rows=ROWS, r_per_part=R):
    key = (rows, r_per_part)
    if key not in _NC_CACHE:
        _NC_CACHE[key] = build_nc(rows, r_per_part)
    return _NC_CACHE[key]


def make_in_maps(parent_states, weight_ih, bias_ih, bias_hh, rows=ROWS):
    parent_states = np.asarray(parent_states, dtype=np.float32)
    weight_ih = np.asarray(weight_ih, dtype=np.float32)
    bias_ih = np.asarray(bias_ih, dtype=np.float32)
    bias_hh = np.asarray(bias_hh, dtype=np.float32)

    w_pre = np.ascontiguousarray(
        weight_ih.reshape(NC12, P, D).transpose(1, 0, 2).reshape(P, NC12 * D)
    )
    bih_pre = np.ascontiguousarray(bias_ih.reshape(NC12, P).T)
    bhh_pre = np.ascontiguousarray(bias_hh.reshape(NC12, P).T)

    in_maps = []
    for c in range(N_CORES):
        in_maps.append(
            {
                "x": parent_states[c * rows : (c + 1) * rows],
                "w": w_pre,
                "bih": bih_pre,
                "bhh": bhh_pre,
            }
        )
    return in_maps


LAST_RESULTS = None


def kernel(
    parent_states,
    weight_ih,
    weight_hh=None,  # unused: h=0 makes weight_hh @ h vanish
    bias_ih=None,
    bias_hh=None,
    _trace=False,
):
    global LAST_RESULTS
    from concourse import bass_utils

    nc = _get_nc()
    in_maps = make_in_maps(parent_states, weight_ih, bias_ih, bias_hh)
    kwargs = {}
    if _trace:
        kwargs = dict(trace=True, trace_cores=[0])
    res = bass_utils.run_bass_kernel_spmd(
        nc, in_maps, core_ids=list(range(N_CORES)), **kwargs
    )
    LAST_RESULTS = res
    hout = np.asarray(res.results[0]["hout"])  # (128, 4)
    return np.ascontiguousarray(hout.T).reshape(H).astype(np.float32)


# revision 4
# speedup vs baseline: 1.1370x; 1.1370x over previous
"""Trainium2 Bass kernel for NodeUpdateGRU: mean over 200000 parent states
followed by a GRUCell update with h=0.

Math (h = 0 simplifies the torch GRUCell):
    agg  = mean(parent_states, axis=0)                  # (512,)
    gi   = weight_ih @ agg + bias_ih                    # (1536,)
    gh   = bias_hh                                      # weight_hh @ 0 == 0
    r    = sigmoid(gi_r + gh_r)
    z    = sigmoid(gi_z + gh_z)
    n    = tanh(gi_n + r * gh_n)
    h'   = (1 - z) * n

Distribution: shard parent_states by rows across 8 cores (contiguous
25000-row slices). Each core computes a partial column sum with the tensor
engine (ones^T @ tile accumulated in PSUM), scales by 1/200000, then a 2KB
AllReduce produces the global mean on every core. The GRU epilogue runs
redundantly on every core; core 0's output is returned.
"""

import sys

for _p in (
    "/root/.axon_site",
    "/root/.axon_site/_ro/trn_rl_repo",
    "/root/.axon_site/_ro/pypackages",
    "/opt/trn_rl_repo",
    "/opt/pypackages",
):
    if _p not in sys.path:
        sys.path.append(_p)

import numpy as np

N_CORES = 8
N_ROWS_TOTAL = 200000
D = 512              # input dim
H = 512              # hidden dim
P = 128              # SBUF partitions
ROWS = N_ROWS_TOTAL // N_CORES   # 25000 rows per core
R = 15               # rows per partition per streaming group
GROUP_ROWS = P * R   # 1920
NC12 = (3 * H) // P  # 12 column-chunks of the 1536-row weight matrix


def build_nc(rows=ROWS, r_per_part=R):
    from concourse import bacc, mybir, tile

    f32 = mybir.dt.float32
    group_rows = P * r_per_part
    n_groups = rows // group_rows
    ragged = rows - n_groups * group_rows
    assert ragged < P

    nc = bacc.Bacc(
        "TRN2", target_bir_lowering=False, debug=False, num_devices=N_CORES
    )

    x = nc.dram_tensor("x", [rows, D], f32, kind="ExternalInput")
    # weight_ih pre-arranged on host: w[p, c*D + d] = W[c*128 + p, d]
    w = nc.dram_tensor("w", [P, NC12 * D], f32, kind="ExternalInput")
    # biases pre-arranged on host: b[p, c] = bias[c*128 + p]
    bih = nc.dram_tensor("bih", [P, NC12], f32, kind="ExternalInput")
    bhh = nc.dram_tensor("bhh", [P, NC12], f32, kind="ExternalInput")
    # output h' arranged as hout[p, c] = h[c*128 + p]
    hout = nc.dram_tensor("hout", [P, 4], f32, kind="ExternalOutput")

    sig = mybir.ActivationFunctionType.Sigmoid
    tanh = mybir.ActivationFunctionType.Tanh
    copy_f = mybir.ActivationFunctionType.Copy

    with tile.TileContext(nc) as tc:
        with (
            tc.tile_pool(name="stream", bufs=3) as stream_pool,
            tc.tile_pool(name="consts", bufs=1) as consts,
            tc.tile_pool(name="psum", bufs=1, space="PSUM") as psum_pool,
            tc.tile_pool(name="dram", bufs=1, space="DRAM") as dram_pool,
            tc.tile_pool(name="small", bufs=2) as small,
        ):
            ones = consts.tile([P, 1], f32)
            nc.vector.memset(ones[:], 1.0)
            ones8 = consts.tile([8, P], f32)
            nc.vector.memset(ones8[:], 1.0)

            # ---- streaming phase: column sums ----
            # fp32 matmuls run at half rate (hi/lo passes), so split each
            # group between the tensor engine (PE_SLICES matmuls into PSUM)
            # and the vector engine (adds into an SBUF accumulator that is
            # folded into PSUM at the end).
            PE_SLICES = min(9, r_per_part)
            psum_acc = psum_pool.tile([1, D], f32)
            acc = consts.tile([P, D], f32)
            nc.vector.memset(acc[:], 0.0)
            mm_i = 0
            for g in range(n_groups):
                t = stream_pool.tile([P, r_per_part * D], f32, tag="xtile")
                src = x[g * group_rows : (g + 1) * group_rows, :].rearrange(
                    "(p t) d -> p (t d)", p=P
                )
                nc.sync.dma_start(out=t[:], in_=src)
                for s in range(r_per_part):
                    if s < PE_SLICES:
                        nc.tensor.matmul(
                            out=psum_acc[:, :],
                            lhsT=ones[:, :],
                            rhs=t[:, s * D : (s + 1) * D],
                            start=(mm_i == 0),
                            stop=False,
                        )
                        mm_i += 1
                    else:
                        nc.vector.tensor_add(
                            out=acc[:], in0=acc[:], in1=t[:, s * D : (s + 1) * D]
                        )
            if ragged:
                t = stream_pool.tile([P, r_per_part * D], f32, tag="xtile")
                nc.sync.dma_start(
                    out=t[:ragged, 0:D], in_=x[n_groups * group_rows : rows, :]
                )
                nc.tensor.matmul(
                    out=psum_acc[:, :],
                    lhsT=ones[:ragged, :],
                    rhs=t[:ragged, 0:D],
                    start=(mm_i == 0),
                    stop=False,
                )
                mm_i += 1
            # fold the DVE accumulator into the PSUM column sums
            nc.tensor.matmul(
                out=psum_acc[:, :],
                lhsT=ones[:, :],
                rhs=acc[:],
                start=False,
                stop=True,
            )

            # weights/biases for the epilogue (loads overlap streaming)
            w_sb = consts.tile([P, NC12 * D], f32)
            nc.sync.dma_start(out=w_sb[:], in_=w[:, :])
            bih_sb = consts.tile([P, NC12], f32)
            nc.sync.dma_start(out=bih_sb[:], in_=bih[:, :])
            bhh_sb = consts.tile([P, NC12], f32)
            nc.sync.dma_start(out=bhh_sb[:], in_=bhh[:, :])
            # bsum: bias init for gi. r/z chunks get bias_ih + bias_hh; n
            # chunks get bias_ih only (bias_hh_n is gated by r later).
            bsum = consts.tile([P, NC12], f32)
            nc.vector.tensor_copy(out=bsum[:], in_=bih_sb[:])
            nc.vector.tensor_add(
                out=bsum[:, 0:8], in0=bsum[:, 0:8], in1=bhh_sb[:, 0:8]
            )

            # local partial mean (scale by 1/N while copying out of PSUM)
            partial = small.tile([1, D], f32)
            nc.scalar.activation(
                out=partial[:],
                in_=psum_acc[:],
                func=copy_f,
                scale=1.0 / float(N_ROWS_TOTAL),
            )

            # ---- cross-core reduction: AllGather (7 ring steps vs 14 for
            # AllReduce) + local sum-with-broadcast via one matmul ----
            cc_in = dram_pool.tile([1, D], f32)
            cc_out = dram_pool.tile([N_CORES, D], f32)
            nc.gpsimd.dma_start(out=cc_in[:], in_=partial[:])
            nc.gpsimd.collective_compute(
                "AllGather",
                mybir.AluOpType.bypass,
                replica_groups=[list(range(N_CORES))],
                ins=[cc_in[:].opt()],
                outs=[cc_out[:].opt()],
            )
            ag_sb = small.tile([N_CORES, D], f32)
            nc.sync.dma_start(out=ag_sb[:], in_=cc_out[:])
            # ones8^T @ ag_sb: sums the 8 partials AND replicates the result
            # onto all 128 partitions in one matmul
            psum_b = psum_pool.tile([P, D], f32)
            nc.tensor.matmul(
                out=psum_b[:, :], lhsT=ones8[:, :], rhs=ag_sb[:, :],
                start=True, stop=True,
            )
            agg_b = consts.tile([P, D], f32)
            nc.scalar.activation(out=agg_b[:], in_=psum_b[:], func=copy_f)

            # ---- GRU epilogue ----
            # gi[:, c] = sum_d W[c*128+p, d] * agg[d] + bsum[p, c]
            # (tensor_tensor_reduce would fuse this but crashes the DVE on
            # this runtime build — use mul + reduce instead)
            gi = small.tile([P, NC12], f32)
            for c in range(NC12):
                prod = small.tile([P, D], f32, tag="prod")
                nc.vector.tensor_mul(
                    out=prod[:], in0=w_sb[:, c * D : (c + 1) * D], in1=agg_b[:]
                )
                nc.vector.reduce_sum(
                    out=gi[:, c : c + 1], in_=prod[:], axis=mybir.AxisListType.X
                )
            nc.vector.tensor_add(out=gi[:], in0=gi[:], in1=bsum[:])

            rz = small.tile([P, 8], f32)
            nc.scalar.activation(out=rz[:], in_=gi[:, 0:8], func=sig)
            pre_n = small.tile([P, 4], f32)
            nc.vector.tensor_mul(out=pre_n[:], in0=rz[:, 0:4], in1=bhh_sb[:, 8:12])
            nc.vector.tensor_add(out=pre_n[:], in0=pre_n[:], in1=gi[:, 8:12])
            n_t = small.tile([P, 4], f32)
            nc.scalar.activation(out=n_t[:], in_=pre_n[:], func=tanh)
            zn = small.tile([P, 4], f32)
            nc.vector.tensor_mul(out=zn[:], in0=rz[:, 4:8], in1=n_t[:])
            hres = small.tile([P, 4], f32)
            nc.vector.tensor_sub(out=hres[:], in0=n_t[:], in1=zn[:])
            nc.sync.dma_start(out=hout[:, :], in_=hres[:])

    nc.compile()
    return nc


_NC_CACHE = {}


def _get_nc(rows=ROWS, r_per_part=R):
    key = (rows, r_per_part)
    if key not in _NC_CACHE:
        _NC_CACHE[key] = build_nc(rows, r_per_part)
    return _NC_CACHE[key]


def make_in_maps(parent_states, weight_ih, bias_ih, bias_hh, rows=ROWS):
    parent_states = np.asarray(parent_states, dtype=np.float32)
    weight_ih = np.asarray(weight_ih, dtype=np.float32)
    bias_ih = np.asarray(bias_ih, dtype=np.float32)
    bias_hh = np.asarray(bias_hh, dtype=np.float32)

    w_pre = np.ascontiguousarray(
        weight_ih.reshape(NC12, P, D).transpose(1, 0, 2).reshape(P, NC12 * D)
    )
    bih_pre = np.ascontiguousarray(bias_ih.reshape(NC12, P).T)
    bhh_pre = np.ascontiguousarray(bias_hh.reshape(NC12, P).T)

    in_maps = []
    for c in range(N_CORES):
        in_maps.append(
            {
                "x": parent_states[c * rows : (c + 1) * rows],
                "w": w_pre,
                "bih": bih_pre,
                "bhh": bhh_pre,
            }
        )
    return in_maps


LAST_RESULTS = None


def kernel(
    parent_states,
    weight_ih,
    weight_hh=None,  # unused: h=0 makes weight_hh @ h vanish
    bias_ih=None,
    bias_hh=None,
    _trace=False,
):
    global LAST_RESULTS
    from concourse import bass_utils

    nc = _get_nc()
    in_maps = make_in_maps(parent_states, weight_ih, bias_ih, bias_hh)
    kwargs = {}
    if _trace:
        kwargs = dict(trace=True, trace_cores=[0])
    res = bass_utils.run_bass_kernel_spmd(
        nc, in_maps, core_ids=list(range(N_CORES)), **kwargs
    )
    LAST_RESULTS = res
    hout = np.asarray(res.results[0]["hout"])  # (128, 4)
    return np.ascontiguousarray(hout.T).reshape(H).astype(np.float32)


# revision 7
# speedup vs baseline: 1.2489x; 1.0984x over previous
"""Trainium2 Bass kernel for NodeUpdateGRU: mean over 200000 parent states
followed by a GRUCell update with h=0.

Math (h = 0 simplifies the torch GRUCell):
    agg  = mean(parent_states, axis=0)                  # (512,)
    gi   = weight_ih @ agg + bias_ih                    # (1536,)
    gh   = bias_hh                                      # weight_hh @ 0 == 0
    r    = sigmoid(gi_r + gh_r)
    z    = sigmoid(gi_z + gh_z)
    n    = tanh(gi_n + r * gh_n)
    h'   = (1 - z) * n

Distribution: shard parent_states by rows across 8 cores (contiguous
25000-row slices). Each core computes a partial column sum with the tensor
engine (ones^T @ tile accumulated in PSUM), scales by 1/200000, then a 2KB
AllReduce produces the global mean on every core. The GRU epilogue runs
redundantly on every core; core 0's output is returned.
"""

import sys

for _p in (
    "/root/.axon_site",
    "/root/.axon_site/_ro/trn_rl_repo",
    "/root/.axon_site/_ro/pypackages",
    "/opt/trn_rl_repo",
    "/opt/pypackages",
):
    if _p not in sys.path:
        sys.path.append(_p)

import numpy as np

N_CORES = 8
N_ROWS_TOTAL = 200000
D = 512              # input dim
H = 512              # hidden dim
P = 128              # SBUF partitions
ROWS = N_ROWS_TOTAL // N_CORES   # 25000 rows per core
R = 15               # rows per partition per streaming group
GROUP_ROWS = P * R   # 1920
NC12 = (3 * H) // P  # 12 column-chunks of the 1536-row weight matrix


def _group_slices(total_slices, first=3, chunk=16):
    """Split `total_slices` row-slices into DMA groups: a small first group
    so compute starts early, then `chunk`-slice groups."""
    groups = [first]
    left = total_slices - first
    while left > 0:
        take = min(chunk, left)
        groups.append(take)
        left -= take
    return groups


def build_nc(rows=ROWS, r_per_part=R):
    from concourse import bacc, mybir, tile

    f32 = mybir.dt.float32
    bf16 = mybir.dt.bfloat16
    n_slices = rows // P          # 512-wide row-slices of 128 rows each
    ragged = rows - n_slices * P  # leftover rows (< 128)
    groups = _group_slices(n_slices)
    max_chunk = max(groups)

    nc = bacc.Bacc(
        "TRN2", target_bir_lowering=False, debug=False, num_devices=N_CORES
    )

    x = nc.dram_tensor("x", [rows, D], f32, kind="ExternalInput")
    # weight_ih^T pre-arranged on host (bf16):
    # wt[p, (k*12 + jb)*128 + m] = W[jb*128 + m, k*128 + p]
    wt = nc.dram_tensor("wt", [P, 48 * P], bf16, kind="ExternalInput")
    # biases pre-arranged on host: b[p, c] = bias[c*128 + p]
    bih = nc.dram_tensor("bih", [P, NC12], f32, kind="ExternalInput")
    bhh = nc.dram_tensor("bhh", [P, NC12], f32, kind="ExternalInput")
    # output h' arranged as hout[p, c] = h[c*128 + p]
    hout = nc.dram_tensor("hout", [P, 4], f32, kind="ExternalOutput")
    # collective bounce buffers (Shared output unlocks the fast path)
    cc_in = nc.dram_tensor("cc_in", [1, D], f32)
    cc_out = nc.dram_tensor("cc_out", [N_CORES, D], f32, addr_space="Shared")

    sig = mybir.ActivationFunctionType.Sigmoid
    tanh = mybir.ActivationFunctionType.Tanh
    copy_f = mybir.ActivationFunctionType.Copy

    with tile.TileContext(nc) as tc:
        with (
            tc.tile_pool(name="stream", bufs=3) as stream_pool,
            tc.tile_pool(name="consts", bufs=1) as consts,
            tc.tile_pool(name="psum", bufs=1, space="PSUM") as psum_pool,
            tc.tile_pool(name="small", bufs=2) as small,
        ):
            ones = consts.tile([P, 1], f32)
            nc.vector.memset(ones[:], 1.0)
            ones8 = consts.tile([8, 1], f32)
            nc.vector.memset(ones8[:], 1.0)

            # ---- streaming phase: column sums ----
            # fp32 matmuls run at half rate (hi/lo passes), so split each
            # group between the tensor engine (PE matmuls into PSUM) and the
            # vector engine (adds into an SBUF accumulator folded into PSUM
            # at the end).
            psum_acc = psum_pool.tile([1, D], f32)
            acc = consts.tile([P, D], f32)
            nc.vector.memset(acc[:], 0.0)
            mm_i = 0
            row0 = 0
            for g, g_slices in enumerate(groups):
                g_rows = g_slices * P
                t = stream_pool.tile([P, max_chunk * D], f32, tag="xtile")
                src = x[row0 : row0 + g_rows, :].rearrange(
                    "(p t) d -> p (t d)", p=P
                )
                nc.sync.dma_start(out=t[:, 0 : g_slices * D], in_=src)
                row0 += g_rows
                pe_slices = g_slices if g_slices <= 9 else 9
                for s in range(g_slices):
                    if s < pe_slices:
                        nc.tensor.matmul(
                            out=psum_acc[:, :],
                            lhsT=ones[:, :],
                            rhs=t[:, s * D : (s + 1) * D],
                            start=(mm_i == 0),
                            stop=False,
                        )
                        mm_i += 1
                    else:
                        nc.vector.tensor_add(
                            out=acc[:], in0=acc[:], in1=t[:, s * D : (s + 1) * D]
                        )
            if ragged:
                t = stream_pool.tile([P, max_chunk * D], f32, tag="xtile")
                nc.sync.dma_start(out=t[:ragged, 0:D], in_=x[row0:rows, :])
                nc.tensor.matmul(
                    out=psum_acc[:, :],
                    lhsT=ones[:ragged, :],
                    rhs=t[:ragged, 0:D],
                    start=(mm_i == 0),
                    stop=False,
                )
                mm_i += 1
            # fold the DVE accumulator into the PSUM column sums
            nc.tensor.matmul(
                out=psum_acc[:, :], lhsT=ones[:, :], rhs=acc[:],
                start=False, stop=True,
            )

            # weights/biases for the epilogue (loads overlap streaming)
            wt_sb = consts.tile([P, 48 * P], bf16)
            nc.sync.dma_start(out=wt_sb[:], in_=wt[:, :])
            bih_sb = consts.tile([P, NC12], f32)
            nc.sync.dma_start(out=bih_sb[:], in_=bih[:, :])
            bhh_sb = consts.tile([P, NC12], f32)
            nc.sync.dma_start(out=bhh_sb[:], in_=bhh[:, :])
            # bsum: bias init for gi. r/z chunks get bias_ih + bias_hh; n
            # chunks get bias_ih only (bias_hh_n is gated by r later).
            bsum = consts.tile([P, NC12], f32)
            nc.vector.tensor_copy(out=bsum[:], in_=bih_sb[:])
            nc.vector.tensor_add(
                out=bsum[:, 0:8], in0=bsum[:, 0:8], in1=bhh_sb[:, 0:8]
            )

            # local partial mean (scale by 1/N while copying out of PSUM)
            partial = small.tile([1, D], f32)
            nc.scalar.activation(
                out=partial[:],
                in_=psum_acc[:],
                func=copy_f,
                scale=1.0 / float(N_ROWS_TOTAL),
            )

            # ---- cross-core reduction: AllGather of the 2KB partials ----
            nc.gpsimd.dma_start(out=cc_in[:, :], in_=partial[:])
            nc.gpsimd.collective_compute(
                "AllGather",
                mybir.AluOpType.bypass,
                replica_groups=[list(range(N_CORES))],
                ins=[cc_in.ap().opt()],
                outs=[cc_out.ap().opt()],
            )
            ag_sb = small.tile([N_CORES, D], f32)
            nc.sync.dma_start(out=ag_sb[:], in_=cc_out[:, :])

            # agg_c[p, k] = sum_c ag_sb[c, k*128 + p]  (global mean, laid out
            # across partitions in 4 column-chunks for the W^T matmuls)
            psum_c = psum_pool.tile([P, 4], f32)
            for k in range(4):
                nc.tensor.matmul(
                    out=psum_c[:, k : k + 1],
                    lhsT=ag_sb[:, k * P : (k + 1) * P],
                    rhs=ones8[:, :],
                    start=True, stop=True,
                )
            agg_c = small.tile([P, 4], bf16)
            nc.scalar.activation(out=agg_c[:], in_=psum_c[:], func=copy_f)

            # ---- GRU epilogue on the tensor engine (bf16 W^T) ----
            # gi[m + 128*jb] = sum_k sum_p W^T[k*128+p, jb*128+m] * agg[k*128+p]
            psum_gi = psum_pool.tile([P, NC12], f32)
            for jb in range(NC12):
                for k in range(4):
                    col = (k * NC12 + jb) * P
                    nc.tensor.matmul(
                        out=psum_gi[:, jb : jb + 1],
                        lhsT=wt_sb[:, col : col + P],
                        rhs=agg_c[:, k : k + 1],
                        start=(k == 0),
                        stop=(k == 3),
                    )
            gi = small.tile([P, NC12], f32)
            nc.vector.tensor_add(out=gi[:], in0=psum_gi[:], in1=bsum[:])

            rz = small.tile([P, 8], f32)
            nc.scalar.activation(out=rz[:], in_=gi[:, 0:8], func=sig)
            pre_n = small.tile([P, 4], f32)
            nc.vector.tensor_mul(out=pre_n[:], in0=rz[:, 0:4], in1=bhh_sb[:, 8:12])
            nc.vector.tensor_add(out=pre_n[:], in0=pre_n[:], in1=gi[:, 8:12])
            n_t = small.tile([P, 4], f32)
            nc.scalar.activation(out=n_t[:], in_=pre_n[:], func=tanh)
            zn = small.tile([P, 4], f32)
            nc.vector.tensor_mul(out=zn[:], in0=rz[:, 4:8], in1=n_t[:])
            hres = small.tile([P, 4], f32)
            nc.vector.tensor_sub(out=hres[:], in0=n_t[:], in1=zn[:])
            nc.sync.dma_start(out=hout[:, :], in_=hres[:])

    nc.compile()
    return nc


_NC_CACHE = {}


def _get_nc(rows=ROWS, r_per_part=R):
    key = (rows, r_per_part)
    if key not in _NC_CACHE:
        _NC_CACHE[key] = build_nc(rows, r_per_part)
    return _NC_CACHE[key]


def make_in_maps(parent_states, weight_ih, bias_ih, bias_hh, rows=ROWS):
    import ml_dtypes

    parent_states = np.asarray(parent_states, dtype=np.float32)
    weight_ih = np.asarray(weight_ih, dtype=np.float32)
    bias_ih = np.asarray(bias_ih, dtype=np.float32)
    bias_hh = np.asarray(bias_hh, dtype=np.float32)

    # W^T tiles for the PE epilogue: wt[p, (k*12+jb)*128+m] = W[jb*128+m, k*128+p]
    wt_pre = np.ascontiguousarray(
        weight_ih.T.reshape(4, P, NC12, P)
        .transpose(1, 0, 2, 3)
        .reshape(P, 48 * P)
        .astype(ml_dtypes.bfloat16)
    )
    bih_pre = np.ascontiguousarray(bias_ih.reshape(NC12, P).T)
    bhh_pre = np.ascontiguousarray(bias_hh.reshape(NC12, P).T)

    in_maps = []
    for c in range(N_CORES):
        in_maps.append(
            {
                "x": parent_states[c * rows : (c + 1) * rows],
                "wt": wt_pre,
                "bih": bih_pre,
                "bhh": bhh_pre,
            }
        )
    return in_maps


LAST_RESULTS = None


def kernel(
    parent_states,
    weight_ih,
    weight_hh=None,  # unused: h=0 makes weight_hh @ h vanish
    bias_ih=None,
    bias_hh=None,
    _trace=False,
):
    global LAST_RESULTS
    from concourse import bass_utils

    nc = _get_nc()
    in_maps = make_in_maps(parent_states, weight_ih, bias_ih, bias_hh)
    kwargs = {}
    if _trace:
        kwargs = dict(trace=True, trace_cores=[0])
    res = bass_utils.run_bass_kernel_spmd(
        nc, in_maps, core_ids=list(range(N_CORES)), **kwargs
    )
    LAST_RESULTS = res
    hout = np.asarray(res.results[0]["hout"])  # (128, 4)
    return np.ascontiguousarray(hout.T).reshape(H).astype(np.float32)
